# revision 1
# baseline (speedup 1.0000x reference)
"""Causal self-attention (B=8, T=1024, C=768, H=12) on 8 TRN2 NeuronCores.

Data-parallel over batch: each core computes one batch element end-to-end
(qkv projection, causal attention, output projection). No collectives.

Self-contained: builds and compiles the Bass program on first call and
caches it for subsequent calls.
"""

import numpy as np

import concourse.bass as bass
import concourse.mybir as mybir
from concourse import bacc
from concourse.tile import TileContext
from concourse.bass_utils import run_bass_kernel_spmd
from concourse.masks import make_identity, make_upper_triangular

f32 = mybir.dt.float32
f32r = mybir.dt.float32r
EXP = mybir.ActivationFunctionType.Exp
COPY = mybir.ActivationFunctionType.Copy

N_CORES = 8
T = 1024          # sequence length (per core batch element)
C = 768           # embedding dim
H = 12            # heads
DK = 64           # head dim
NCC = C // 128    # 6 C-chunks
NTT = T // 128    # 8 token tiles
SCALE = 1.0 / np.sqrt(DK)


def _r(ap):
    return ap if ap.dtype == f32r else ap.bitcast(f32r)


def build_program(qkv_bias: bool, out_bias: bool, iters: int = 1):
    nc = bacc.Bacc("TRN2", num_devices=N_CORES, debug=False)

    x = nc.dram_tensor("x", [T, C], f32, kind="ExternalInput").ap()
    wqkv = nc.dram_tensor("W_qkv", [C, 3 * C], f32r, kind="ExternalInput").ap()
    bqkv = nc.dram_tensor("b_qkv", [3 * C], f32r, kind="ExternalInput").ap()
    wout = nc.dram_tensor("W_out", [C, C], f32r, kind="ExternalInput").ap()
    bout = nc.dram_tensor("b_out", [C], f32r, kind="ExternalInput").ap()
    y = nc.dram_tensor("y", [T, C], f32, kind="ExternalOutput").ap()

    with TileContext(nc) as tc:
        with tc.tile_pool(name="const", bufs=1) as cpool, \
             tc.tile_pool(name="qk", bufs=1) as qkpool, \
             tc.tile_pool(name="vpp", bufs=1) as vppool, \
             tc.tile_pool(name="attn", bufs=1) as atpool:

            ident = cpool.tile([128, 128], f32, tag="ident")
            make_identity(nc, ident)

            # Triangle mask (1 where k<=q within a diagonal 128x128 block)
            tri = cpool.tile([128, 128], f32, tag="tri")
            make_upper_triangular(nc, tri, val=1.0, diag=True)

            if qkv_bias or out_bias:
                ones_row = cpool.tile([1, 512], f32r, tag="ones_row")
                nc.gpsimd.memset(ones_row, 1.0)
            if qkv_bias:
                bq_sb = cpool.tile([1, 3 * C], f32r, tag="bq")
                nc.sync.dma_start(out=bq_sb, in_=bqkv[None, :])
            if out_bias:
                bo_sb = cpool.tile([1, C], f32r, tag="bo")
                nc.sync.dma_start(out=bo_sb, in_=bout[None, :])

            for it in range(iters):
                # Persistent activations (slots shared across bench iterations)
                qkT = [qkpool.tile([128, T], f32r, tag=f"qkT{m}", name=f"qkT{m}") for m in range(12)]
                vp = [vppool.tile([128, H * 65], f32r, tag=f"vp{t}", name=f"vp{t}") for t in range(NTT)]

                # ---------------- Phase A+B: xT, qkT, V' ----------------
                with tc.tile_pool(name=f"xload{it}", bufs=6) as xpool, \
                     tc.tile_pool(name=f"wq{it}", bufs=1) as wqpool, \
                     tc.tile_pool(name=f"xT{it}", bufs=1) as xTpool, \
                     tc.tile_pool(name=f"psA{it}", bufs=3, space="PSUM") as psA, \
                     tc.tile_pool(name=f"psB{it}", bufs=5, space="PSUM") as psB:

                    xT = [xTpool.tile([128, T], f32r, tag=f"xT{c}", name=f"xT{c}") for c in range(NCC)]
                    wq = [wqpool.tile([128, 3 * C], f32r, tag=f"wq{c}", name=f"wq{c}")
                          for c in range(NCC)]
                    for t in range(NTT):
                        xt = xpool.tile([128, C], f32, tag="x")
                        dma = nc.gpsimd if t % 2 else nc.sync
                        dma.dma_start(out=xt, in_=x[t * 128:(t + 1) * 128, :])
                        for c in range(NCC):
                            tp = psA.tile([128, 128], f32, tag="tp")
                            nc.tensor.transpose(tp, xt[:, c * 128:(c + 1) * 128], ident)
                            nc.scalar.activation(
                                xT[c][:, t * 128:(t + 1) * 128], tp, COPY)
                    for c in range(NCC):
                        # split each chunk across both DMA queues to halve latency
                        nc.gpsimd.dma_start(out=wq[c][:, 0:1152],
                                            in_=wqkv[c * 128:(c + 1) * 128, 0:1152])
                        nc.sync.dma_start(out=wq[c][:, 1152:],
                                          in_=wqkv[c * 128:(c + 1) * 128, 1152:])

                    # q, k feature-major: qkT[m] rows = features m*128..m*128+127
                    # (m 0..5 -> q features 0..767, m 6..11 -> k features 0..767)
                    for m in (0, 6, 1, 7, 2, 8, 3, 9, 4, 10, 5, 11):
                        for nj in range(2):
                            ps = psB.tile([128, 512], f32, tag="mm")
                            if qkv_bias:
                                nc.tensor.matmul(
                                    ps, _r(bq_sb[0:1, m * 128:(m + 1) * 128]),
                                    _r(ones_row), start=True, stop=False)
                            for c in range(NCC):
                                nc.tensor.matmul(
                                    ps,
                                    _r(wq[c][:, m * 128:(m + 1) * 128]),
                                    _r(xT[c][:, nj * 512:(nj + 1) * 512]),
                                    start=(c == 0 and not qkv_bias), stop=(c == NCC - 1))
                            nc.vector.tensor_copy(qkT[m][:, nj * 512:(nj + 1) * 512], ps)

                    # v token-major, written into V' with a ones column per head
                    for t in range(NTT):
                        for n0, nw in ((0, 512), (512, 256)):
                            ps = psB.tile([128, 512], f32, tag="mm")
                            psv = ps[:, 0:nw]
                            if qkv_bias:
                                nc.tensor.matmul(
                                    psv, _r(ones_row[0:1, 0:128]),
                                    _r(bq_sb[0:1, 2 * C + n0:2 * C + n0 + nw]),
                                    start=True, stop=False)
                            for c in range(NCC):
                                nc.tensor.matmul(
                                    psv,
                                    _r(xT[c][:, t * 128:(t + 1) * 128]),
                                    _r(wq[c][:, 2 * C + n0:2 * C + n0 + nw]),
                                    start=(c == 0 and not qkv_bias), stop=(c == NCC - 1))
                            dst = vp[t].rearrange("p (h e) -> p h e", e=65)[
                                :, n0 // 64:(n0 + nw) // 64, 0:64]
                            nc.vector.tensor_copy(
                                dst, psv.rearrange("p (a b) -> p a b", b=64))
                        nc.gpsimd.memset(
                            vp[t].rearrange("p (h e) -> p h e", e=65)[:, :, 64:65]
                            .bitcast(f32), 1.0)

                # ---------------- Phase C: attention ----------------
                with tc.tile_pool(name=f"pb{it}", bufs=8) as pbpool, \
                     tc.tile_pool(name=f"rr{it}", bufs=6) as rrpool, \
                     tc.tile_pool(name=f"wo{it}", bufs=1) as wopool:
                  woutT = [wopool.tile([128, C], f32r, tag=f"woutT{c}",
                                       name=f"woutT{c}") for c in range(NCC)]
                  for c in range(NCC):
                      # W_out loads overlap attention on the idle gpsimd queue
                      nc.gpsimd.dma_start(out=woutT[c],
                                          in_=wout[c * 128:(c + 1) * 128, :])
                  with tc.tile_pool(name=f"psS{it}", bufs=3, space="PSUM") as psS, \
                       tc.tile_pool(name=f"psPV{it}", bufs=2, space="PSUM") as psPV:

                    attnT = [atpool.tile([128, T], f32r, tag=f"attnT{c}",
                                         name=f"attnT{c}") for c in range(NCC)]
                    for hp in range(H // 2):
                        # head pair (2hp, 2hp+1): partitions 0:64 / 64:128 of the
                        # same qkT tiles -> S matmuls land in different PE row
                        # groups and overlap on the array.
                        qTt = qkT[hp]
                        kTt = qkT[6 + hp]
                        for qj in range(2):
                            nki = 4 * qj + 4
                            pvs = [psPV.tile([128, 512], f32, tag="pv",
                                             name=f"pv{hp}_{qj}_{e}") for e in range(2)]
                            for g in range(nki // 2):
                                sps = [psS.tile([128, 1024], f32, tag="s",
                                                name=f"s{hp}_{qj}_{g}_{e}")
                                       for e in range(2)]
                                jp0 = g - 2 * qj
                                # per-ki placement in the 1024-col group:
                                # (colbase, o) with o = causally-dead prefix
                                # width that is never computed. The near-band
                                # group stores [j1|j0] so its single dead
                                # prefix sits at the group start and exp can
                                # run as one suffix op.
                                if jp0 == 0:
                                    placement = [(512, 0), (0, 128)]
                                elif jp0 == 1:
                                    # j3's true prefix is 384, but N=128 runs
                                    # at 1/4 fp32r rate (= N=256 cost); widen
                                    # to N=256 - the extra columns are never
                                    # read (exp skips them, pb is zeroed)
                                    placement = [(0, 256), (512, 256)]
                                else:
                                    placement = [(0, 0), (512, 0)]
                                for loc in range(2):
                                    ki = g * 2 + loc
                                    cb, o = placement[loc]
                                    for e in range(2):
                                        qb = e * 64
                                        nc.tensor.matmul(
                                            sps[e][:, cb + o:cb + 512],
                                            _r(kTt[qb:qb + 64,
                                                   ki * 128:(ki + 1) * 128]),
                                            _r(qTt[qb:qb + 64,
                                                   qj * 512 + o:(qj + 1) * 512]),
                                            start=True, stop=True)
                                jp = jp0
                                pbs = []
                                for e in range(2):
                                    pb = pbpool.tile([128, 1024], f32r, tag="pb",
                                                     name=f"pb{hp}_{qj}_{g}_{e}")
                                    if jp == 1:
                                        # one strided op over both valid-ish
                                        # 256-col blocks ([256:512) and
                                        # [768:1024)); the dead [768:896) part
                                        # is finite and memset-zeroed after
                                        nc.scalar.activation(
                                            pb.rearrange("p (a b) -> p a b",
                                                         b=256)[:, 1:4:2, :],
                                            sps[e].rearrange("p (a b) -> p a b",
                                                             b=256)[:, 1:4:2, :],
                                            EXP, scale=float(SCALE))
                                    elif jp == 0:
                                        nc.scalar.activation(
                                            pb[:, 128:1024], sps[e][:, 128:1024],
                                            EXP, scale=float(SCALE))
                                    else:
                                        nc.scalar.activation(pb, sps[e], EXP,
                                                             scale=float(SCALE))
                                    pbs.append(pb)
                                if jp == 0:
                                    # layout [j1|j0]: zero the j1 prefix,
                                    # triangles at [128:256) (j1) and
                                    # [512:640) (j0)
                                    for e in range(2):
                                        pb = pbs[e]
                                        nc.gpsimd.memset(
                                            pb[:, 0:128].bitcast(f32), 0.0)
                                        nc.vector.tensor_mul(
                                            pb[:, 128:256], pb[:, 128:256], tri)
                                        nc.vector.tensor_mul(
                                            pb[:, 512:640], pb[:, 512:640], tri)
                                elif jp == 1:
                                    for e in range(2):
                                        pb = pbs[e]
                                        nc.gpsimd.memset(
                                            pb[:, 0:256].bitcast(f32), 0.0)
                                        nc.vector.tensor_mul(
                                            pb[:, 256:384], pb[:, 256:384], tri)
                                        nc.gpsimd.memset(
                                            pb[:, 512:896].bitcast(f32), 0.0)
                                        nc.vector.tensor_mul(
                                            pb[:, 896:1024], pb[:, 896:1024], tri)
                                # PV: stream only valid columns, clamped to
                                # N>=256 (below that fp32r runs at 1/4 rate so
                                # narrower costs the same). Emit in placement
                                # order with ki ascending per column region so
                                # the start=True (ki==0) matmul executes first.
                                for loc in range(2):
                                    ki = g * 2 + loc
                                    cb, o = placement[loc]
                                    ov = min(o, 256)
                                    for e in range(2):
                                        h = 2 * hp + e
                                        nc.tensor.matmul(
                                            pvs[e][0:65, ov:512],
                                            _r(vp[ki][:, h * 65:(h + 1) * 65]),
                                            _r(pbs[e][:, cb + ov:cb + 512]),
                                            start=(ki == 0), stop=(ki == nki - 1))
                            for e in range(2):
                                qb = e * 64
                                # copy PV out of PSUM promptly to release the
                                # bank for the next group's matmuls
                                pvsb = rrpool.tile([65, 512], f32, tag="pvsb",
                                                   name=f"pvsb{hp}_{qj}_{e}")
                                nc.vector.tensor_copy(pvsb, pvs[e][0:65, :])
                                recip = rrpool.tile([1, 512], f32, tag="recip",
                                                    name=f"recip{hp}_{qj}_{e}")
                                nc.vector.reciprocal(recip, pvsb[64:65, :])
                                rb = rrpool.tile([64, 512], f32, tag="rb",
                                                 name=f"rb{hp}_{qj}_{e}")
                                nc.gpsimd.partition_broadcast(rb, recip)
                                nc.vector.tensor_mul(
                                    attnT[hp][qb:qb + 64, qj * 512:(qj + 1) * 512],
                                    pvsb[0:64, :], rb)

                  # -------------- Phase D: output projection --------------
                  with tc.tile_pool(name=f"yst{it}", bufs=4) as ypool, \
                       tc.tile_pool(name=f"psO{it}", bufs=4, space="PSUM") as psO:
                    for t in range(NTT):
                        for n0, nw in ((0, 512), (512, 256)):
                            ps = psO.tile([128, 512], f32, tag="o")
                            pso = ps[:, 0:nw]
                            if out_bias:
                                nc.tensor.matmul(
                                    pso, _r(ones_row[0:1, 0:128]),
                                    _r(bo_sb[0:1, n0:n0 + nw]), start=True, stop=False)
                            for c in range(NCC):
                                nc.tensor.matmul(
                                    pso,
                                    _r(attnT[c][:, t * 128:(t + 1) * 128]),
                                    _r(woutT[c][:, n0:n0 + nw]),
                                    start=(c == 0 and not out_bias), stop=(c == NCC - 1))
                            ysb = ypool.tile([128, 512], f32, tag="y")
                            nc.vector.tensor_copy(ysb[:, 0:nw], pso)
                            nc.sync.dma_start(
                                out=y[t * 128:(t + 1) * 128, n0:n0 + nw],
                                in_=ysb[:, 0:nw])

    nc.compile()
    return nc


_CACHE = {}


def _get_program(qkv_bias: bool, out_bias: bool):
    key = (qkv_bias, out_bias)
    if key not in _CACHE:
        _CACHE[key] = build_program(qkv_bias, out_bias)
    return _CACHE[key]


def _make_in_maps(x, W_qkv, b_qkv, W_out, b_out):
    x = np.ascontiguousarray(np.asarray(x, dtype=np.float32))
    W_qkv = np.ascontiguousarray(np.asarray(W_qkv, dtype=np.float32))
    b_qkv = np.ascontiguousarray(np.asarray(b_qkv, dtype=np.float32))
    W_out = np.ascontiguousarray(np.asarray(W_out, dtype=np.float32))
    b_out = np.ascontiguousarray(np.asarray(b_out, dtype=np.float32))
    return [
        {"x": x[i], "W_qkv": W_qkv, "b_qkv": b_qkv, "W_out": W_out, "b_out": b_out}
        for i in range(N_CORES)
    ]


def kernel(x, W_qkv, b_qkv, W_out, b_out):
    qkv_bias = bool(np.any(np.asarray(b_qkv)))
    out_bias = bool(np.any(np.asarray(b_out)))
    nc = _get_program(qkv_bias, out_bias)
    in_maps = _make_in_maps(x, W_qkv, b_qkv, W_out, b_out)
    res = run_bass_kernel_spmd(nc, in_maps, core_ids=list(range(N_CORES)))
    return np.stack([res.results[i]["y"] for i in range(N_CORES)], axis=0)


def bench(x, W_qkv, b_qkv, W_out, b_out, trace=True):
    """Run with NTFF tracing; returns (output, BassKernelResults)."""
    qkv_bias = bool(np.any(np.asarray(b_qkv)))
    out_bias = bool(np.any(np.asarray(b_out)))
    nc = _get_program(qkv_bias, out_bias)
    in_maps = _make_in_maps(x, W_qkv, b_qkv, W_out, b_out)
    res = run_bass_kernel_spmd(nc, in_maps, core_ids=list(range(N_CORES)),
                               trace=trace)
    out = np.stack([res.results[i]["y"] for i in range(N_CORES)], axis=0)
    return out, res

